# revision 19
# baseline (speedup 1.0000x reference)
"""Trainium2 Bass kernel for nn_AttentionModule (multi-head attention pooling).

Math per sample n (N=16384, SPLIT=100, INPUT_DIM=128, H=4 heads, PER_DIM=64):
  xs = x.reshape(n, 100, 128)
  h[s, (hd,o)] = xs[s, :] @ W[hd][:, o]          (projection, out 256 cols)
  score[s, hd] = leaky_relu(h[s, hd*64:] . q[hd], 0.2) = xs[s,:] . wq[hd]
  att = softmax_s(score);  out[n] = sum_s att[s,hd] * h[s, (hd,o)]

Strategy (data-parallel over 8 cores, 2048 samples each):
  - the host (inside kernel(), as input prep) casts x to bf16 and lays it out
    pre-transposed as xT[nblk, S, I, BLK]: each [I, BLK] tile is the
    transposed x slice a matmul needs as its stationary operand.  The device
    then runs NO transposes, NO casts, NO transpose-evicts, and reads half
    the HBM bytes.
  - per split s: one matmul, stationary xT tile [i, n], moving
    Wb = [W | wq] bf16 [128, 260] -> h+score [n=128p, 260] fp32 PSUM.
  - h evict (fp32 PSUM -> bf16 SBUF, layout [n, c, s]) on ACT; score evict
    (fp32 [n, 4, s]) on DVE.
  - the block's 100 splits form two independent halves (48+52) with per-half
    h/score/u tiles; each half's pooling runs parallel to the other's evicts.
  - softmax without max subtraction (scores are O(6), exp safe in f32).
  - pooling on DVE: tensor_tensor multiply with stride-0 broadcast of u over
    the 64 per-head dims (2x_1P), then a pairwise bf16 fold tree down to
    width 3 and one small reduce; normalize by 1/sum at the end.  The tail is
    emitted in SMALL CHUNKS interleaved (in program order) with later groups'
    PSUM-critical evicts, so the strict per-engine FIFO never parks a
    bank-recycle evict behind a multi-microsecond pooling burst.
"""

import sys

if "/opt/trn_rl_repo" not in sys.path:
    sys.path.insert(0, "/opt/trn_rl_repo")

import numpy as np

N_TOTAL = 16384
NCORES = 8
S = 100
I = 128
H = 4
O = 64
OUT = 256
COLS = 260  # 256 projection cols + 4 score cols
BLK = 128
SG = 4  # splits per PSUM group
HALF = 48  # first-half split count (must be a multiple of SG)
CHUNKS_PER_GROUP = 2  # pending tail chunks drained after each group

_BUILT = {}


def build_bass(npc):
    """Build the per-core Bass program for npc samples (npc % 128 == 0)."""
    import concourse.bass as bass
    import concourse.mybir as mybir
    from concourse import bacc
    from concourse.bass import broadcast_tensor_aps
    from concourse.tile import TileContext

    dt = mybir.dt
    nblk = npc // BLK
    nc = bacc.Bacc()

    xTd = nc.declare_dram_parameter(
        "xT", [nblk, S, I, BLK], dt.bfloat16, isOutput=False
    )
    Wd = nc.declare_dram_parameter("W", [H, I, O], dt.float32, isOutput=False)
    qd = nc.declare_dram_parameter("q", [H, O], dt.float32, isOutput=False)
    od = nc.declare_dram_parameter("out", [npc, OUT], dt.float32, isOutput=True)

    groups = []
    s0 = 0
    while s0 < S:
        groups.append((s0, min(SG, S - s0)))
        s0 += SG
    halves = ((0, HALF), (HALF, S))

    with TileContext(nc) as tc:
        with (
            tc.tile_pool(name="const", bufs=1) as cpool,
            tc.tile_pool(name="xtp", bufs=6) as xtp,
            tc.tile_pool(name="hwp", bufs=2) as hwp,
            tc.tile_pool(name="smp", bufs=2) as smp,
            tc.tile_pool(name="outp", bufs=2) as outp,
            tc.tile_pool(name="redp", bufs=2) as redp,
            tc.tile_pool(name="php", bufs=2, space="PSUM") as php,
        ):
            # ---- setup: Wb = [W | wq] bf16, staged on DVE (matmuls read the
            # xT tiles via DMA semaphores, so Wb is their only engine dep)
            Wf = cpool.tile([128, H, O], dt.float32)  # [i, hd, o]
            nc.sync.dma_start(out=Wf[:, :, :], in_=Wd[:, :, :].rearrange("h i o -> i h o"))
            W2s = cpool.tile([O, H, I], dt.float32)  # [o, hd, i]
            nc.sync.dma_start(out=W2s[:, :, :], in_=Wd[:, :, :].rearrange("h i o -> o h i"))
            q2s = cpool.tile([O, H], dt.float32)  # [o, hd]
            nc.sync.dma_start(out=q2s[:, :], in_=qd[:, :].rearrange("h o -> o h"))
            W2 = cpool.tile([O, H, I], dt.float32)
            nc.vector.tensor_copy(out=W2[:, :, :], in_=W2s[:, :, :])
            q2 = cpool.tile([O, H], dt.float32)
            nc.vector.tensor_copy(out=q2[:, :], in_=q2s[:, :])

            wqp = php.tile([128, H], dt.float32, tag="ph")
            for hd in range(H):
                # wq[:, hd] -- lhsT=[o,(i)], rhs=[o,1] -> out [i,1]
                nc.tensor.matmul(
                    wqp[:, hd : hd + 1],
                    lhsT=W2[:, hd, :],
                    rhs=q2[:, hd : hd + 1],
                    start=True,
                    stop=True,
                )
            Wb = cpool.tile([128, COLS], dt.bfloat16)
            nc.vector.tensor_copy(
                out=Wb[:, 0:OUT].rearrange("p (h o) -> p h o", h=H), in_=Wf[:, :, :]
            )
            nc.vector.tensor_copy(out=Wb[:, OUT:COLS], in_=wqp[:, :])

            def make_final(b, den, prh):
                def f():
                    dsum = smp.tile([128, H], dt.float32, tag="dsum")
                    nc.vector.tensor_tensor(
                        out=dsum[:, :],
                        in0=den[0][:, :],
                        in1=den[1][:, :],
                        op=mybir.AluOpType.add,
                    )
                    rec = smp.tile([128, H], dt.float32, tag="rec")
                    nc.vector.reciprocal(rec[:, :], dsum[:, :])
                    pr = outp.tile([128, OUT], dt.float32, tag="pr")
                    nc.vector.tensor_tensor(
                        out=pr[:, :],
                        in0=prh[0][:, :],
                        in1=prh[1][:, :],
                        op=mybir.AluOpType.add,
                    )
                    of = outp.tile([128, OUT], dt.float32, tag="of")
                    o0 = pr[:, :].rearrange("p (h o) -> p h o", h=H)
                    o1 = rec[:, :].unsqueeze(2)  # [p, h, 1]
                    oo = of[:, :].rearrange("p (h o) -> p h o", h=H)
                    o0b, o1b = broadcast_tensor_aps(o0, o1)
                    nc.vector.tensor_tensor(
                        out=oo, in0=o0b, in1=o1b, op=mybir.AluOpType.mult
                    )
                    nc.sync.dma_start(
                        out=od[b * BLK : (b + 1) * BLK, :], in_=of[:, :]
                    )
                return f

            pending = []
            for b in range(nblk):
                # per-half tiles: half k's pooling chain is independent of
                # half k+1's evicts
                hw = [
                    hwp.tile([128, OUT, hi - lo], dt.bfloat16, tag=f"hw{k}", name=f"hw{k}")
                    for k, (lo, hi) in enumerate(halves)
                ]
                scs = [
                    smp.tile([128, H, hi - lo], dt.float32, tag=f"sc{k}", name=f"sc{k}")
                    for k, (lo, hi) in enumerate(halves)
                ]
                t1 = [
                    smp.tile([128, H, hi - lo], dt.float32, tag=f"t1{k}", name=f"t1{k}")
                    for k, (lo, hi) in enumerate(halves)
                ]
                uf = [
                    smp.tile([128, H, hi - lo], dt.float32, tag=f"uf{k}", name=f"uf{k}")
                    for k, (lo, hi) in enumerate(halves)
                ]
                ub = [
                    smp.tile([128, H, hi - lo], dt.bfloat16, tag=f"ub{k}", name=f"ub{k}")
                    for k, (lo, hi) in enumerate(halves)
                ]
                prh = [
                    redp.tile([128, OUT], dt.float32, tag=f"prh{k}", name=f"prh{k}")
                    for k in range(2)
                ]
                den = [
                    smp.tile([128, H], dt.float32, tag=f"den{k}", name=f"den{k}")
                    for k in range(2)
                ]

                def tail_chunks(k):
                    """Closures emitting the half-k pooling tail in small
                    per-engine chunks (interleaved by the caller)."""
                    lo, hi = halves[k]
                    w = hi - lo
                    hwk, sck, t1k, ufk, ubk = hw[k], scs[k], t1[k], uf[k], ub[k]
                    denk, prhk = den[k], prh[k]
                    out = []

                    def c_leaky():
                        nc.vector.tensor_scalar_mul(t1k[:, :, :], sck[:, :, :], 0.2)
                        nc.vector.tensor_tensor(
                            out=sck[:, :, :],
                            in0=sck[:, :, :],
                            in1=t1k[:, :, :],
                            op=mybir.AluOpType.max,
                        )

                    def c_exp():
                        nc.scalar.activation(
                            out=ufk[:, :, :],
                            in_=sck[:, :, :],
                            func=mybir.ActivationFunctionType.Exp,
                        )

                    def c_ub():
                        nc.vector.tensor_copy(out=ubk[:, :, :], in_=ufk[:, :, :])

                    def c_den():
                        nc.vector.tensor_reduce(
                            out=denk[:, :],
                            in_=ufk[:, :, :],
                            axis=mybir.AxisListType.X,
                            op=mybir.AluOpType.add,
                        )

                    # the u chain is latency-critical: emit it NOW so exp
                    # lands in ACT's FIFO right after this half's sc evicts
                    c_leaky(); c_exp(); c_ub(); c_den()

                    def c_mult(hd):
                        def f():
                            in0 = hwk[:, hd * O : (hd + 2) * O, :].rearrange(
                                "p (h o) s -> p h o s", h=2
                            )
                            in1 = ubk[:, hd : hd + 2, :].unsqueeze(2)
                            in0b, in1b = broadcast_tensor_aps(in0, in1)
                            nc.vector.tensor_tensor(
                                out=in0b, in0=in0b, in1=in1b,
                                op=mybir.AluOpType.mult,
                            )
                        return f

                    out += [c_mult(0), c_mult(2)]

                    # fold plan down to width 3
                    plan = []
                    ww = w
                    while ww > 3:
                        a2 = ww // 2
                        plan.append((ww, a2))
                        ww = a2

                    def c_fold(a2, c0, c1):
                        def f():
                            with nc.allow_low_precision("bf16 partial sums"):
                                nc.vector.tensor_tensor(
                                    out=hwk[:, c0:c1, 0:a2],
                                    in0=hwk[:, c0:c1, 0:a2],
                                    in1=hwk[:, c0:c1, a2 : 2 * a2],
                                    op=mybir.AluOpType.add,
                                )
                        return f

                    leftovers = []
                    for wv, a2 in plan:
                        out.append(c_fold(a2, 0, 256))
                        if wv % 2:
                            leftovers.append(2 * a2)

                    def c_reduce():
                        nc.vector.tensor_reduce(
                            out=prhk[:, :],
                            in_=hwk[:, :, 0:ww],
                            axis=mybir.AxisListType.X,
                            op=mybir.AluOpType.add,
                        )
                        for col in leftovers:
                            nc.vector.tensor_tensor(
                                out=prhk[:, :],
                                in0=prhk[:, :],
                                in1=hwk[:, :, col],
                                op=mybir.AluOpType.add,
                            )

                    out.append(c_reduce)
                    return out

                for gi, (s0, ns) in enumerate(groups):
                    k = 0 if s0 < HALF else 1
                    lo = halves[k][0]
                    xt = xtp.tile([128, SG, 128], dt.bfloat16, tag="xt")
                    ph = php.tile([128, SG, 512], dt.float32, tag="ph")
                    # one DMA fetches the group's pre-transposed bf16 tiles
                    nc.sync.dma_start(
                        out=xt[:, 0:ns, :],
                        in_=xTd[b, s0 : s0 + ns, :, :].rearrange("s i n -> i s n"),
                    )
                    for j in range(ns):
                        nc.tensor.matmul(
                            ph[:, j, 0:COLS],
                            lhsT=xt[:, j, :],
                            rhs=Wb[:, :],
                            start=True,
                            stop=True,
                        )
                    # evict: h -> hw[k] (bf16, [n, c, s]) on ACT, score on DVE
                    nc.scalar.copy(
                        out=hw[k][:, :, s0 - lo : s0 - lo + ns],
                        in_=ph[:, 0:ns, 0:OUT].rearrange("p s c -> p c s"),
                    )
                    nc.scalar.copy(
                        out=scs[k][:, :, s0 - lo : s0 - lo + ns],
                        in_=ph[:, 0:ns, OUT:COLS].rearrange("p s h -> p h s"),
                    )
                    if s0 + ns == HALF:
                        pending.extend(tail_chunks(0))
                    elif s0 + ns == S:
                        pending.extend(tail_chunks(1))
                        pending.append(make_final(b, den, prh))
                    for _ in range(CHUNKS_PER_GROUP):
                        if pending:
                            pending.pop(0)()
            while pending:
                pending.pop(0)()

    nc.finalize()
    return nc


def _get(npc):
    if npc not in _BUILT:
        _BUILT[npc] = build_bass(npc)
    return _BUILT[npc]


def _prep_xT(xc):
    """[npc, 12800] fp32 -> bf16 tiles [nblk, S, I, BLK] (transposed)."""
    import ml_dtypes

    npc = xc.shape[0]
    nblk = npc // BLK
    xb = xc.astype(ml_dtypes.bfloat16)
    xb = xb.reshape(nblk, BLK, S, I).transpose(0, 2, 3, 1)  # [nblk, S, I, BLK]
    return np.ascontiguousarray(xb)


def kernel(x, W, q, _trace=False):
    x = np.ascontiguousarray(np.asarray(x, dtype=np.float32))
    W = np.ascontiguousarray(np.asarray(W, dtype=np.float32))
    q = np.ascontiguousarray(np.asarray(q, dtype=np.float32))
    n = x.shape[0]
    npc = n // NCORES
    nc = _get(npc)

    from concourse.bass_utils import run_bass_kernel_spmd

    in_maps = [
        {"xT": _prep_xT(x[c * npc : (c + 1) * npc]), "W": W, "q": q}
        for c in range(NCORES)
    ]
    res = run_bass_kernel_spmd(
        nc, in_maps, core_ids=list(range(NCORES)), trace=_trace
    )
    out = np.concatenate([res.results[c]["out"] for c in range(NCORES)], axis=0)
    if _trace:
        return out.astype(np.float32), res
    return out.astype(np.float32)


# revision 20
# speedup vs baseline: 1.2775x; 1.2775x over previous
"""Trainium2 Bass kernel for nn_AttentionModule (multi-head attention pooling).

Math per sample n (N=16384, SPLIT=100, INPUT_DIM=128, H=4 heads, PER_DIM=64):
  xs = x.reshape(n, 100, 128)
  h[s, (hd,o)] = xs[s, :] @ W[hd][:, o]          (projection, out 256 cols)
  score[s, hd] = leaky_relu(h[s, hd*64:] . q[hd], 0.2) = xs[s,:] . wq[hd]
  att = softmax_s(score);  out[n] = sum_s att[s,hd] * h[s, (hd,o)]

Strategy (data-parallel over 8 cores, 2048 samples each):
  - the host (inside kernel(), as input prep) casts x to bf16 and lays it out
    pre-transposed as xT[nblk, S, I, BLK]: each [I, BLK] tile is the
    transposed x slice a matmul needs as its stationary operand.  The device
    then runs NO transposes, NO casts, NO transpose-evicts, and reads half
    the HBM bytes.
  - per split s: one matmul, stationary xT tile [i, n], moving
    Wb = [W | wq] bf16 [128, 260] -> h+score [n=128p, 260] fp32 PSUM.
  - h evict (fp32 PSUM -> bf16 SBUF, layout [n, c, s]) on ACT; score evict
    (fp32 [n, 4, s]) on DVE.
  - the block's 100 splits form two independent halves (48+52) with per-half
    h/score/u tiles; each half's pooling runs parallel to the other's evicts.
  - softmax without max subtraction (scores are O(6), exp safe in f32).
  - pooling on DVE: tensor_tensor multiply with stride-0 broadcast of u over
    the 64 per-head dims (2x_1P), then a pairwise bf16 fold tree down to
    width 3 and one small reduce; normalize by 1/sum at the end.  The tail is
    emitted in SMALL CHUNKS interleaved (in program order) with later groups'
    PSUM-critical evicts, so the strict per-engine FIFO never parks a
    bank-recycle evict behind a multi-microsecond pooling burst.
"""

import sys

if "/opt/trn_rl_repo" not in sys.path:
    sys.path.insert(0, "/opt/trn_rl_repo")

import numpy as np

N_TOTAL = 16384
NCORES = 8
S = 100
I = 128
H = 4
O = 64
OUT = 256
COLS = 260  # 256 projection cols + 4 score cols
BLK = 128
SG = 4  # splits per PSUM group
HALF = 48  # first-half split count (must be a multiple of SG)
CHUNKS_PER_GROUP = 2  # pending tail chunks drained after each group

_BUILT = {}


def build_bass(npc):
    """Build the per-core Bass program for npc samples (npc % 128 == 0)."""
    import concourse.bass as bass
    import concourse.mybir as mybir
    from concourse import bacc
    from concourse.bass import broadcast_tensor_aps
    from concourse.tile import TileContext

    dt = mybir.dt
    nblk = npc // BLK
    nc = bacc.Bacc()

    xTd = nc.declare_dram_parameter(
        "xT", [nblk, S, I, BLK], dt.bfloat16, isOutput=False
    )
    Wd = nc.declare_dram_parameter("W", [H, I, O], dt.float32, isOutput=False)
    qd = nc.declare_dram_parameter("q", [H, O], dt.float32, isOutput=False)
    od = nc.declare_dram_parameter("out", [npc, OUT], dt.float32, isOutput=True)

    groups = []
    s0 = 0
    while s0 < S:
        groups.append((s0, min(SG, S - s0)))
        s0 += SG
    halves = ((0, HALF), (HALF, S))

    with TileContext(nc) as tc:
        with (
            tc.tile_pool(name="const", bufs=1) as cpool,
            tc.tile_pool(name="xtp", bufs=6) as xtp,
            tc.tile_pool(name="hwp", bufs=2) as hwp,
            tc.tile_pool(name="smp", bufs=2) as smp,
            tc.tile_pool(name="outp", bufs=2) as outp,
            tc.tile_pool(name="redp", bufs=2) as redp,
            tc.tile_pool(name="php", bufs=2, space="PSUM") as php,
        ):
            # ---- setup: Wb = [W | wq] bf16, staged on DVE (matmuls read the
            # xT tiles via DMA semaphores, so Wb is their only engine dep)
            Wf = cpool.tile([128, H, O], dt.float32)  # [i, hd, o]
            nc.sync.dma_start(out=Wf[:, :, :], in_=Wd[:, :, :].rearrange("h i o -> i h o"))
            W2s = cpool.tile([O, H, I], dt.float32)  # [o, hd, i]
            nc.sync.dma_start(out=W2s[:, :, :], in_=Wd[:, :, :].rearrange("h i o -> o h i"))
            q2s = cpool.tile([O, H], dt.float32)  # [o, hd]
            nc.sync.dma_start(out=q2s[:, :], in_=qd[:, :].rearrange("h o -> o h"))
            W2 = cpool.tile([O, H, I], dt.float32)
            nc.vector.tensor_copy(out=W2[:, :, :], in_=W2s[:, :, :])
            q2 = cpool.tile([O, H], dt.float32)
            nc.vector.tensor_copy(out=q2[:, :], in_=q2s[:, :])

            wqp = php.tile([128, H], dt.float32, tag="ph")
            for hd in range(H):
                # wq[:, hd] -- lhsT=[o,(i)], rhs=[o,1] -> out [i,1]
                nc.tensor.matmul(
                    wqp[:, hd : hd + 1],
                    lhsT=W2[:, hd, :],
                    rhs=q2[:, hd : hd + 1],
                    start=True,
                    stop=True,
                )
            Wb = cpool.tile([128, COLS], dt.bfloat16)
            nc.vector.tensor_copy(
                out=Wb[:, 0:OUT].rearrange("p (h o) -> p h o", h=H), in_=Wf[:, :, :]
            )
            nc.vector.tensor_copy(out=Wb[:, OUT:COLS], in_=wqp[:, :])

            def make_final(b, den, prh):
                def f():
                    dsum = smp.tile([128, H], dt.float32, tag="dsum")
                    nc.vector.tensor_tensor(
                        out=dsum[:, :],
                        in0=den[0][:, :],
                        in1=den[1][:, :],
                        op=mybir.AluOpType.add,
                    )
                    rec = smp.tile([128, H], dt.float32, tag="rec")
                    nc.vector.reciprocal(rec[:, :], dsum[:, :])
                    pr = outp.tile([128, OUT], dt.float32, tag="pr")
                    nc.vector.tensor_tensor(
                        out=pr[:, :],
                        in0=prh[0][:, :],
                        in1=prh[1][:, :],
                        op=mybir.AluOpType.add,
                    )
                    of = outp.tile([128, OUT], dt.float32, tag="of")
                    o0 = pr[:, :].rearrange("p (h o) -> p h o", h=H)
                    o1 = rec[:, :].unsqueeze(2)  # [p, h, 1]
                    oo = of[:, :].rearrange("p (h o) -> p h o", h=H)
                    o0b, o1b = broadcast_tensor_aps(o0, o1)
                    nc.vector.tensor_tensor(
                        out=oo, in0=o0b, in1=o1b, op=mybir.AluOpType.mult
                    )
                    nc.sync.dma_start(
                        out=od[b * BLK : (b + 1) * BLK, :], in_=of[:, :]
                    )
                return f

            pending = []
            for b in range(nblk):
                # per-half tiles: half k's pooling chain is independent of
                # half k+1's evicts
                hw = [
                    hwp.tile([128, COLS, hi - lo], dt.bfloat16, tag=f"hw{k}", name=f"hw{k}")
                    for k, (lo, hi) in enumerate(halves)
                ]
                t1 = [
                    smp.tile([128, H, hi - lo], dt.bfloat16, tag=f"t1{k}", name=f"t1{k}")
                    for k, (lo, hi) in enumerate(halves)
                ]
                ub = [
                    smp.tile([128, H, hi - lo], dt.bfloat16, tag=f"ub{k}", name=f"ub{k}")
                    for k, (lo, hi) in enumerate(halves)
                ]
                prh = [
                    redp.tile([128, OUT], dt.float32, tag=f"prh{k}", name=f"prh{k}")
                    for k in range(2)
                ]
                den = [
                    smp.tile([128, H], dt.float32, tag=f"den{k}", name=f"den{k}")
                    for k in range(2)
                ]

                def tail_chunks(k):
                    """Closures emitting the half-k pooling tail in small
                    per-engine chunks (interleaved by the caller)."""
                    lo, hi = halves[k]
                    w = hi - lo
                    hwk, t1k, ubk = hw[k], t1[k], ub[k]
                    sck = hwk[:, OUT:COLS, :]
                    denk, prhk = den[k], prh[k]
                    out = []

                    def c_leaky():
                        nc.vector.tensor_scalar_mul(t1k[:, :, :], sck, 0.2)
                        nc.vector.tensor_tensor(
                            out=sck,
                            in0=sck,
                            in1=t1k[:, :, :],
                            op=mybir.AluOpType.max,
                        )

                    def c_exp():
                        nc.scalar.activation(
                            out=ubk[:, :, :],
                            in_=sck,
                            func=mybir.ActivationFunctionType.Exp,
                        )

                    def c_den():
                        nc.vector.tensor_reduce(
                            out=denk[:, :],
                            in_=ubk[:, :, :],
                            axis=mybir.AxisListType.X,
                            op=mybir.AluOpType.add,
                        )

                    # the u chain is latency-critical: emit it NOW so exp
                    # lands in ACT's FIFO right after this half's evicts
                    c_leaky(); c_exp(); c_den()

                    def c_mult(hd):
                        def f():
                            in0 = hwk[:, hd * O : (hd + 2) * O, :].rearrange(
                                "p (h o) s -> p h o s", h=2
                            )
                            in1 = ubk[:, hd : hd + 2, :].unsqueeze(2)
                            in0b, in1b = broadcast_tensor_aps(in0, in1)
                            nc.vector.tensor_tensor(
                                out=in0b, in0=in0b, in1=in1b,
                                op=mybir.AluOpType.mult,
                            )
                        return f

                    out += [c_mult(0), c_mult(2)]

                    # fold plan down to width 3
                    plan = []
                    ww = w
                    while ww > 3:
                        a2 = ww // 2
                        plan.append((ww, a2))
                        ww = a2

                    def c_fold(a2, c0, c1):
                        def f():
                            with nc.allow_low_precision("bf16 partial sums"):
                                nc.vector.tensor_tensor(
                                    out=hwk[:, c0:c1, 0:a2],
                                    in0=hwk[:, c0:c1, 0:a2],
                                    in1=hwk[:, c0:c1, a2 : 2 * a2],
                                    op=mybir.AluOpType.add,
                                )
                        return f

                    leftovers = []
                    for wv, a2 in plan:
                        out.append(c_fold(a2, 0, 256))
                        if wv % 2:
                            leftovers.append(2 * a2)

                    def c_reduce():
                        nc.vector.tensor_reduce(
                            out=prhk[:, :],
                            in_=hwk[:, 0:OUT, 0:ww],
                            axis=mybir.AxisListType.X,
                            op=mybir.AluOpType.add,
                        )
                        for col in leftovers:
                            nc.vector.tensor_tensor(
                                out=prhk[:, :],
                                in0=prhk[:, :],
                                in1=hwk[:, 0:OUT, col],
                                op=mybir.AluOpType.add,
                            )

                    out.append(c_reduce)
                    return out

                for gi, (s0, ns) in enumerate(groups):
                    k = 0 if s0 < HALF else 1
                    lo = halves[k][0]
                    xt = xtp.tile([128, SG, 128], dt.bfloat16, tag="xt")
                    ph = php.tile([128, SG, 512], dt.float32, tag="ph")
                    # one DMA fetches the group's pre-transposed bf16 tiles
                    nc.sync.dma_start(
                        out=xt[:, 0:ns, :],
                        in_=xTd[b, s0 : s0 + ns, :, :].rearrange("s i n -> i s n"),
                    )
                    for j in range(ns):
                        nc.tensor.matmul(
                            ph[:, j, 0:COLS],
                            lhsT=xt[:, j, :],
                            rhs=Wb[:, :],
                            start=True,
                            stop=True,
                        )
                    # one evict: h+score -> hw[k] (bf16, [n, c, s]) on ACT
                    nc.scalar.copy(
                        out=hw[k][:, :, s0 - lo : s0 - lo + ns],
                        in_=ph[:, 0:ns, 0:COLS].rearrange("p s c -> p c s"),
                    )
                    if s0 + ns == HALF:
                        pending.extend(tail_chunks(0))
                    elif s0 + ns == S:
                        pending.extend(tail_chunks(1))
                        pending.append(make_final(b, den, prh))
                    for _ in range(CHUNKS_PER_GROUP):
                        if pending:
                            pending.pop(0)()
            while pending:
                pending.pop(0)()

    nc.finalize()
    return nc


def _get(npc):
    if npc not in _BUILT:
        _BUILT[npc] = build_bass(npc)
    return _BUILT[npc]


def _prep_xT(xc):
    """[npc, 12800] fp32 -> bf16 tiles [nblk, S, I, BLK] (transposed)."""
    import ml_dtypes

    npc = xc.shape[0]
    nblk = npc // BLK
    xb = xc.astype(ml_dtypes.bfloat16)
    xb = xb.reshape(nblk, BLK, S, I).transpose(0, 2, 3, 1)  # [nblk, S, I, BLK]
    return np.ascontiguousarray(xb)


def kernel(x, W, q, _trace=False):
    x = np.ascontiguousarray(np.asarray(x, dtype=np.float32))
    W = np.ascontiguousarray(np.asarray(W, dtype=np.float32))
    q = np.ascontiguousarray(np.asarray(q, dtype=np.float32))
    n = x.shape[0]
    npc = n // NCORES
    nc = _get(npc)

    from concourse.bass_utils import run_bass_kernel_spmd

    in_maps = [
        {"xT": _prep_xT(x[c * npc : (c + 1) * npc]), "W": W, "q": q}
        for c in range(NCORES)
    ]
    res = run_bass_kernel_spmd(
        nc, in_maps, core_ids=list(range(NCORES)), trace=_trace
    )
    out = np.concatenate([res.results[c]["out"] for c in range(NCORES)], axis=0)
    if _trace:
        return out.astype(np.float32), res
    return out.astype(np.float32)
